# revision 30
# baseline (speedup 1.0000x reference)
"""Trainium2 Bass kernel for nn_Attention_41137196761104.

Dense transformer attention block:
    qkv = x @ Wqkv + bqkv            (B=2, N=2048, D=2048, H=16, HD=128)
    k,v = concat(prefix, k/v)        (PREFIX=512, KT=2560)   [cached pre-rope]
    q,k = rope(q), rope(k)
    out = softmax(q k^T / sqrt(HD), causal+prefix mask) v @ Wff + bff
Returns (out [B,N,D], next_prefix_kv [2,B,H,KT,HD]).

Sharding: 8 cores = batch(2) x head-groups(4).  Each core computes 4 heads
of one batch element: column-parallel QKV, per-head attention, row-parallel
Wff producing a partial output.  Host sums the 4 partials per batch and
assembles the kv cache (prefix slices come directly from the inputs).

Per-core schedule (engine-overlap driven):
  1. v projection first, token chunks streaming x from HBM (x stays
     resident afterwards); v is token-major so PV stationaries and the
     v-cache output are both natural.
  2. q/k projection head-outer (q0,k0,q1,k1,...), token-chunk-inner with
     4 live PSUM banks; each head's in-place rope (DVE) chases its
     evictions and hides under the remaining projection matmuls.
  3. Attention chunk-outer/head-inner: scoresT[kt,q] via kT-stationary
     matmuls (no transposes anywhere); softmax over kt(partitions) with
     the denominator computed by an all-ones [128,128] stationary matmul
     accumulating a broadcast column-sum in PSUM; reciprocal_approx_fast;
     the FF projection for each finished chunk chases behind.
  4. Causal structure hardcoded: invalid kt tiles skipped, the 4
     diagonal-band tiles per chunk masked with static patterns.
All matmuls bf16 with f32 PSUM accumulation (end-to-end rel err ~7e-3).
"""

import math
import os

import ml_dtypes
import numpy as np

B, N, D, H, PRE = 2, 2048, 2048, 16, 512
HD = 128
KTOK = N + PRE          # 2560 key tokens
KT = KTOK // 128        # 20 kt tiles
NH = 4                  # heads per core
NCORES = 8
ROPE_BASE = 10000.0

_BF16 = ml_dtypes.bfloat16
_CACHE = {}


def _bf(a):
    return np.ascontiguousarray(a, dtype=np.float32).astype(_BF16)


def _f32(a):
    return np.ascontiguousarray(a, dtype=np.float32)


def _rope_tables():
    """cos/sin duplicated across both partition halves (the HW verifier
    requires both TensorTensor SBUF inputs to share a base partition)."""
    inv = 1.0 / (ROPE_BASE ** (np.arange(0, HD, 2, dtype=np.float32) / HD))
    tabs = []
    for n in (N, KTOK):
        f = np.arange(n, dtype=np.float32)[:, None] * inv[None, :]
        cc = np.empty((128, n), np.float32)
        ss = np.empty((128, n), np.float32)
        cc[0:64] = cc[64:128] = np.cos(f).T
        ss[0:64] = ss[64:128] = np.sin(f).T
        tabs += [_bf(cc), _bf(ss)]
    return tabs  # ccq, ssq [128, N], cck, ssk [128, KTOK]


def _band_masks():
    m = np.zeros((128, 4, 512), np.float32)
    i = np.arange(128)[:, None]
    j = np.arange(512)[None, :]
    for p in range(4):
        m[:, p, :] = (128 * p + i <= j).astype(np.float32)
    return _bf(m)


def _build_nc():
    import concourse.bass as bass  # noqa: F401
    import concourse.mybir as mybir
    import concourse.tile as tile
    from concourse import bacc

    F32 = mybir.dt.float32
    BF16 = mybir.dt.bfloat16
    AF = mybir.ActivationFunctionType

    nc = bacc.Bacc(
        "TRN2",
        target_bir_lowering=False,
        debug=False,
        num_devices=NCORES,
    )

    def din(name, shape, dt=BF16):
        return nc.dram_tensor(name, shape, dt, kind="ExternalInput").ap()

    def dout(name, shape, dt=F32):
        return nc.dram_tensor(name, shape, dt, kind="ExternalOutput").ap()

    xT_d = din("xT", [16, 128, N])          # x[b].T, d-tiled
    wqk_d = din("wqk", [16, 128, 1024])     # Wqkv cols [q 4hd | k 4hd], d-tiled
    wv_d = din("wv", [16, 128, 512])        # Wqkv v cols, d-tiled
    wff_d = din("wff", [4, 128, 2048])      # Wff rows for these heads, e-tiled
    bqk_d = din("bqk", [128, 8], F32)       # bias col per f-tile (q 0-3, k 4-7)
    bv_d = din("bv", [128, 512], F32)       # v bias broadcast to all partitions
    pkT_d = din("pkT", [NH, 128, PRE])      # prefix_k^T per head
    pv_d = din("pv", [NH, 4, 128, 128])     # prefix_v, kt-tiled, per head
    ccq_d = din("ccq", [128, N])            # rope tables, cos/sin on all parts
    ssq_d = din("ssq", [128, N])
    cck_d = din("cck", [128, KTOK])
    ssk_d = din("ssk", [128, KTOK])
    mb_d = din("mband", [128, 4, 512])      # diagonal-band causal masks

    out_d = dout("out_p", [16, 128, 2048])  # partial output, t-tiled
    knew_d = dout("knew", [NH, 128, N], BF16)   # pre-rope k^T per head
    vnew_d = dout("vnew", [16, 128, 512], BF16)  # pre-rope v, t-tiled

    HALF = KTOK // 2  # rope scratch works in two free-dim halves

    with tile.TileContext(nc) as tc:
        with (
            tc.tile_pool(name="const", bufs=1) as cpool,
            tc.tile_pool(name="persist", bufs=1) as ppool,
        ):
            bqk = cpool.tile([128, 8], F32)
            bv = cpool.tile([128, 512], F32)

            qT = ppool.tile([128, NH, N], BF16)       # per head [HD, N]
            kTf = ppool.tile([128, NH, KTOK], BF16)   # [prefix | new]
            vv = ppool.tile([128, 16, 512], BF16)     # token-major v, t-tiled
            ones = ppool.tile([128, 128], BF16)       # colsum+broadcast statnry
            nc.any.memset(ones[:], 1.0)

            # ---- Phase 1: projections + rope ----
            with (
                tc.tile_pool(name="xfull", bufs=1) as xp,
                tc.tile_pool(name="ropetab", bufs=1) as rtp,
                tc.tile_pool(name="ropescr", bufs=1) as rsp,
            ):
                xf = xp.tile([128, 16, N], BF16)
                ccq = rtp.tile([128, N], BF16)
                ssq = rtp.tile([128, N], BF16)
                cck = rtp.tile([128, KTOK], BF16)
                ssk = rtp.tile([128, KTOK], BF16)

                def rope_head(src, cc, ss, L, h):
                    # in-place rotate-half rope on src[:, h, 0:L], two
                    # free-dim halves to keep scratch small
                    for j in range((L + HALF - 1) // HALF):
                        w = min(HALF, L - j * HALF)
                        fs = slice(j * HALF, j * HALF + w)
                        sa = rsp.tile([128, HALF], BF16, tag="sa")
                        sb = rsp.tile([128, HALF], BF16, tag="sb")
                        x1 = src[0:64, h, fs]
                        x2 = src[64:128, h, fs]
                        nc.vector.tensor_mul(sa[0:64, :w], x1, cc[0:64, fs])
                        nc.vector.tensor_mul(sa[64:128, :w], x2, cc[64:128, fs])
                        nc.vector.tensor_mul(sb[0:64, :w], x2, ss[64:128, fs])
                        nc.vector.tensor_mul(sb[64:128, :w], x1, ss[0:64, fs])
                        nc.vector.tensor_sub(x1, sa[0:64, :w], sb[0:64, :w])
                        nc.vector.tensor_add(x2, sa[64:128, :w], sb[64:128, :w])

                # (a) v projection, token chunks streaming x
                with (
                    tc.tile_pool(name="wv", bufs=1) as wvp,
                    tc.tile_pool(name="wqk", bufs=1) as wqkp,
                ):
                    wv = wvp.tile([128, 16, 512], BF16)
                    wqk = wqkp.tile([128, 16, 1024], BF16)
                    nc.sync.dma_start(bv[:], bv_d[:])
                    for d in range(16):
                        nc.sync.dma_start(
                            xf[:, d, 0:512], xT_d[d, :, 0:512]
                        )
                        nc.sync.dma_start(wv[:, d, :], wv_d[d])
                    for d in range(16):
                        nc.sync.dma_start(
                            xf[:, d, 512:1024], xT_d[d, :, 512:1024]
                        )
                    # q/k weights early enough that the head-outer phase
                    # never waits on them; x chunks 2-3 still beat their use.
                    nc.sync.dma_start(bqk[:], bqk_d[:])
                    for d in range(16):
                        nc.sync.dma_start(wqk[:, d, :], wqk_d[d])
                    for c in range(2, 4):
                        for d in range(16):
                            nc.sync.dma_start(
                                xf[:, d, c * 512 : (c + 1) * 512],
                                xT_d[d, :, c * 512 : (c + 1) * 512],
                            )
                    nc.sync.dma_start(ccq[:], ccq_d[:])
                    nc.sync.dma_start(ssq[:], ssq_d[:])
                    nc.sync.dma_start(cck[:], cck_d[:])
                    nc.sync.dma_start(ssk[:], ssk_d[:])
                    for h in range(NH):
                        nc.sync.dma_start(kTf[:, h, 0:PRE], pkT_d[h])

                    with tc.tile_pool(name="psv", bufs=4, space="PSUM") as psv:
                        for ti in range(16):
                            ps = psv.tile([128, 512], F32, tag="psv")
                            for d in range(16):
                                nc.tensor.matmul(
                                    ps[:],
                                    xf[:, d, ti * 128 : (ti + 1) * 128],
                                    wv[:, d, :],
                                    start=(d == 0),
                                    stop=(d == 15),
                                )
                            nc.vector.tensor_add(vv[:, ti, :], ps[:], bv[:])
                            nc.sync.dma_start(vnew_d[ti], vv[:, ti, :])

                    # (b) q/k projection, head-outer with rope chasing
                    with tc.tile_pool(name="psqk", bufs=8, space="PSUM") as psqk:
                        for f in (0, 4, 1, 5, 2, 6, 3, 7):  # q0,k0,q1,k1,...
                            pss = [
                                psqk.tile(
                                    [128, 512], F32, tag="psqk", name="psqk"
                                )
                                for _ in range(4)
                            ]
                            for d in range(16):
                                for c in range(4):
                                    nc.tensor.matmul(
                                        pss[c][:],
                                        wqk[:, d, f * 128 : (f + 1) * 128],
                                        xf[:, d, c * 512 : (c + 1) * 512],
                                        start=(d == 0),
                                        stop=(d == 15),
                                    )
                            for c in range(4):
                                cs = slice(c * 512, (c + 1) * 512)
                                if f < 4:
                                    nc.scalar.activation(
                                        qT[:, f, cs], pss[c][:], AF.Identity,
                                        bias=bqk[:, f : f + 1],
                                    )
                                else:
                                    nc.scalar.activation(
                                        kTf[
                                            :, f - 4,
                                            PRE + c * 512 : PRE + (c + 1) * 512,
                                        ],
                                        pss[c][:], AF.Identity,
                                        bias=bqk[:, f : f + 1],
                                    )
                            if f < 4:
                                rope_head(qT, ccq, ssq, N, f)
                            else:
                                h = f - 4
                                nc.sync.dma_start(
                                    knew_d[h], kTf[:, h, PRE:KTOK]
                                )
                                rope_head(kTf, cck, ssk, KTOK, h)

            # ---- Phase 2: attention (chunk-outer) + chased FF projection ----
            inv_sqrt_hd = 1.0 / math.sqrt(HD)
            with (
                tc.tile_pool(name="probs", bufs=2) as prp,
                tc.tile_pool(name="attc", bufs=1) as acp,
                tc.tile_pool(name="rcp", bufs=2) as rcpool,
                tc.tile_pool(name="wff", bufs=1) as wffp,
                tc.tile_pool(name="ost", bufs=4) as ostp,
                tc.tile_pool(name="pssc", bufs=3, space="PSUM") as pssc,
                tc.tile_pool(name="psdv", bufs=3, space="PSUM") as psdv,
                tc.tile_pool(name="ps4", bufs=2, space="PSUM") as ps4,
            ):
                pvs = acp.tile([128, NH, 4, 128], BF16)   # prefix v, kt-tiled
                mband = acp.tile([128, 4, 512], BF16)
                aoT = acp.tile([128, NH, N], BF16)        # attn out, f-major
                wff = wffp.tile([128, 4, 2048], BF16)
                for h in range(NH):
                    for t in range(4):
                        nc.sync.dma_start(pvs[:, h, t, :], pv_d[h, t])
                nc.sync.dma_start(mband[:], mb_d[:])
                for h in range(NH):
                    nc.sync.dma_start(wff[:, h, :], wff_d[h])

                for c in range(4):
                    cs = slice(c * 512, (c + 1) * 512)
                    nvalid = 4 * c + 8
                    for h in range(NH):
                        probs = prp.tile([128, KT, 512], BF16, tag="probs")
                        dn = psdv.tile([128, 512], F32, tag="dnpv")
                        for kt in range(nvalid):
                            ps = pssc.tile([128, 512], F32, tag="pssc")
                            nc.tensor.matmul(
                                ps[:],
                                kTf[:, h, kt * 128 : (kt + 1) * 128],
                                qT[:, h, cs],
                                start=True,
                                stop=True,
                            )
                            nc.scalar.activation(
                                probs[:, kt, :], ps[:], AF.Exp,
                                scale=inv_sqrt_hd,
                            )
                            if kt >= nvalid - 4:
                                p = kt - (nvalid - 4)
                                nc.vector.tensor_mul(
                                    probs[:, kt, :], probs[:, kt, :],
                                    mband[:, p, :],
                                )
                            nc.tensor.matmul(
                                dn[:],
                                ones[:],
                                probs[:, kt, :],
                                start=(kt == 0),
                                stop=(kt == nvalid - 1),
                            )
                        rc = rcpool.tile([128, 512], F32, tag="rc")
                        nc.vector.reciprocal_approx_fast(rc[:], dn[:])
                        po = psdv.tile([128, 512], F32, tag="dnpv")
                        for kt in range(nvalid):
                            vt = (
                                pvs[:, h, kt, :]
                                if kt < 4
                                else vv[:, kt - 4, h * 128 : (h + 1) * 128]
                            )
                            nc.tensor.matmul(
                                po[:],
                                vt,
                                probs[:, kt, :],
                                start=(kt == 0),
                                stop=(kt == nvalid - 1),
                            )
                        nc.vector.tensor_mul(aoT[:, h, cs], po[:], rc[:])
                    # FF projection for this chunk's token tiles (all heads of
                    # chunk c are now available) — overlaps the next chunk.
                    for tt in range(4):
                        ti = 4 * c + tt
                        for oc in range(4):
                            ps = ps4.tile([128, 512], F32, tag="ps4")
                            for h in range(NH):
                                nc.tensor.matmul(
                                    ps[:],
                                    aoT[:, h, ti * 128 : (ti + 1) * 128],
                                    wff[:, h, oc * 512 : (oc + 1) * 512],
                                    start=(h == 0),
                                    stop=(h == 3),
                                )
                            st = ostp.tile([128, 512], F32, tag="ost")
                            nc.scalar.copy(st[:], ps[:])
                            nc.sync.dma_start(
                                out_d[ti, :, oc * 512 : (oc + 1) * 512], st[:]
                            )

    nc.compile()
    return nc


def get_nc():
    if "nc" not in _CACHE:
        _CACHE["nc"] = _build_nc()
    return _CACHE["nc"]


def make_in_maps(x, prefix_k, prefix_v, Wqkv, bqkv, Wff, bff):
    x = _f32(x)
    prefix_k = _f32(prefix_k)
    prefix_v = _f32(prefix_v)
    Wqkv = _f32(Wqkv)
    bqkv = _f32(bqkv)
    Wff = _f32(Wff)
    ccq, ssq, cck, ssk = _rope_tables()
    mband = _band_masks()
    bvb = np.empty((128, 512), np.float32)
    in_maps = []
    for core in range(NCORES):
        b, g = divmod(core, 4)
        f0 = 4 * g * HD
        wq = Wqkv[:, f0 : f0 + 512]
        wk = Wqkv[:, D + f0 : D + f0 + 512]
        wv = Wqkv[:, 2 * D + f0 : 2 * D + f0 + 512]
        bq = bqkv[f0 : f0 + 512]
        bk = bqkv[D + f0 : D + f0 + 512]
        bvv = bqkv[2 * D + f0 : 2 * D + f0 + 512]
        bqk = np.concatenate([bq, bk]).reshape(8, 128).T  # [128, 8]
        bvb[:] = bvv[None, :]
        hs = slice(4 * g, 4 * g + 4)
        in_maps.append(
            {
                "xT": _bf(x[b].T.reshape(16, 128, N)),
                "wqk": _bf(
                    np.concatenate([wq, wk], axis=1).reshape(16, 128, 1024)
                ),
                "wv": _bf(wv.reshape(16, 128, 512)),
                "wff": _bf(Wff[f0 : f0 + 512].reshape(4, 128, 2048)),
                "bqk": _f32(bqk),
                "bv": _f32(bvb),
                "pkT": _bf(prefix_k[b, hs].transpose(0, 2, 1)),
                "pv": _bf(prefix_v[b, hs].reshape(NH, 4, 128, 128)),
                "ccq": ccq,
                "ssq": ssq,
                "cck": cck,
                "ssk": ssk,
                "mband": mband,
            }
        )
    return in_maps


def assemble(results, prefix_k, prefix_v, bff):
    out = np.zeros((B, N, D), np.float32)
    kv = np.empty((2, B, H, KTOK, HD), np.float32)
    kv[0, :, :, :PRE] = prefix_k
    kv[1, :, :, :PRE] = prefix_v
    for core in range(NCORES):
        b, g = divmod(core, 4)
        r = results[core]
        out[b] += r["out_p"].reshape(N, D)
        hs = slice(4 * g, 4 * g + 4)
        kv[0, b, hs, PRE:] = r["knew"].astype(np.float32).transpose(0, 2, 1)
        kv[1, b, hs, PRE:] = (
            r["vnew"].astype(np.float32).reshape(N, NH, HD).transpose(1, 0, 2)
        )
    out += _f32(bff)[None, None, :]
    return out, kv


def kernel(x, mask, prefix_k, prefix_v, Wqkv, bqkv, Wff, bff):
    from concourse.bass_utils import run_bass_kernel_spmd

    nc = get_nc()
    in_maps = make_in_maps(x, prefix_k, prefix_v, Wqkv, bqkv, Wff, bff)
    res = run_bass_kernel_spmd(
        nc,
        in_maps,
        core_ids=list(range(NCORES)),
        trace=bool(int(os.environ.get("KERNEL_TRACE", "0"))),
    )
    _CACHE["last_results"] = res
    return assemble(res.results, _f32(prefix_k), _f32(prefix_v), bff)


# revision 31
# speedup vs baseline: 1.0424x; 1.0424x over previous
"""Trainium2 Bass kernel for nn_Attention_41137196761104.

Dense transformer attention block:
    qkv = x @ Wqkv + bqkv            (B=2, N=2048, D=2048, H=16, HD=128)
    k,v = concat(prefix, k/v)        (PREFIX=512, KT=2560)   [cached pre-rope]
    q,k = rope(q), rope(k)
    out = softmax(q k^T / sqrt(HD), causal+prefix mask) v @ Wff + bff
Returns (out [B,N,D], next_prefix_kv [2,B,H,KT,HD]).

Sharding: 8 cores = batch(2) x head-groups(4).  Each core computes 4 heads
of one batch element: column-parallel QKV, per-head attention, row-parallel
Wff producing a partial output.  Host sums the 4 partials per batch and
assembles the kv cache (prefix slices come directly from the inputs).

Per-core schedule (engine-overlap driven):
  1. v projection first, token chunks streaming x from HBM (x stays
     resident afterwards); v is token-major so PV stationaries and the
     v-cache output are both natural.
  2. q/k projection head-outer (q0,k0,q1,k1,...), token-chunk-inner with
     4 live PSUM banks; each head's in-place rope (DVE) chases its
     evictions and hides under the remaining projection matmuls.
  3. Attention chunk-outer/head-inner: scoresT[kt,q] via kT-stationary
     matmuls (no transposes anywhere); softmax over kt(partitions) with
     the denominator computed by an all-ones [128,128] stationary matmul
     accumulating a broadcast column-sum in PSUM; reciprocal_approx_fast;
     the FF projection for each finished chunk chases behind.
  4. Causal structure hardcoded: invalid kt tiles skipped, the 4
     diagonal-band tiles per chunk masked with static patterns.
All matmuls bf16 with f32 PSUM accumulation (end-to-end rel err ~7e-3).
"""

import math
import os

import ml_dtypes
import numpy as np

B, N, D, H, PRE = 2, 2048, 2048, 16, 512
HD = 128
KTOK = N + PRE          # 2560 key tokens
KT = KTOK // 128        # 20 kt tiles
NH = 4                  # heads per core
NCORES = 8
ROPE_BASE = 10000.0

_BF16 = ml_dtypes.bfloat16
_CACHE = {}


def _bf(a):
    return np.ascontiguousarray(a, dtype=np.float32).astype(_BF16)


def _f32(a):
    return np.ascontiguousarray(a, dtype=np.float32)


def _rope_tables():
    """cos/sin duplicated across both partition halves (the HW verifier
    requires both TensorTensor SBUF inputs to share a base partition)."""
    inv = 1.0 / (ROPE_BASE ** (np.arange(0, HD, 2, dtype=np.float32) / HD))
    tabs = []
    for n in (N, KTOK):
        f = np.arange(n, dtype=np.float32)[:, None] * inv[None, :]
        cc = np.empty((128, n), np.float32)
        ss = np.empty((128, n), np.float32)
        cc[0:64] = cc[64:128] = np.cos(f).T
        ss[0:64] = ss[64:128] = np.sin(f).T
        tabs += [_bf(cc), _bf(ss)]
    return tabs  # ccq, ssq [128, N], cck, ssk [128, KTOK]


def _band_masks():
    m = np.zeros((128, 4, 512), np.float32)
    i = np.arange(128)[:, None]
    j = np.arange(512)[None, :]
    for p in range(4):
        m[:, p, :] = (128 * p + i <= j).astype(np.float32)
    return _bf(m)


def _build_nc():
    import concourse.bass as bass  # noqa: F401
    import concourse.mybir as mybir
    import concourse.tile as tile
    from concourse import bacc

    F32 = mybir.dt.float32
    BF16 = mybir.dt.bfloat16
    AF = mybir.ActivationFunctionType

    nc = bacc.Bacc(
        "TRN2",
        target_bir_lowering=False,
        debug=False,
        num_devices=NCORES,
    )

    def din(name, shape, dt=BF16):
        return nc.dram_tensor(name, shape, dt, kind="ExternalInput").ap()

    def dout(name, shape, dt=F32):
        return nc.dram_tensor(name, shape, dt, kind="ExternalOutput").ap()

    xT_d = din("xT", [16, 128, N])          # x[b].T, d-tiled
    wqk_d = din("wqk", [16, 128, 1024])     # Wqkv cols [q 4hd | k 4hd], d-tiled
    wv_d = din("wv", [16, 128, 512])        # Wqkv v cols, d-tiled
    wff_d = din("wff", [4, 128, 2048])      # Wff rows for these heads, e-tiled
    bqk_d = din("bqk", [128, 8], F32)       # bias col per f-tile (q 0-3, k 4-7)
    bv_d = din("bv", [128, 512], F32)       # v bias broadcast to all partitions
    pkT_d = din("pkT", [NH, 128, PRE])      # prefix_k^T per head
    pv_d = din("pv", [NH, 4, 128, 128])     # prefix_v, kt-tiled, per head
    ccq_d = din("ccq", [128, N])            # rope tables, cos/sin on all parts
    ssq_d = din("ssq", [128, N])
    cck_d = din("cck", [128, KTOK])
    ssk_d = din("ssk", [128, KTOK])
    mb_d = din("mband", [128, 4, 512])      # diagonal-band causal masks

    out_d = dout("out_p", [16, 128, 2048])  # partial output, t-tiled
    knew_d = dout("knew", [NH, 128, N], BF16)   # pre-rope k^T per head
    vnew_d = dout("vnew", [16, 128, 512], BF16)  # pre-rope v, t-tiled

    HALF = KTOK // 2  # rope scratch works in two free-dim halves

    with tile.TileContext(nc) as tc:
        with (
            tc.tile_pool(name="const", bufs=1) as cpool,
            tc.tile_pool(name="persist", bufs=1) as ppool,
        ):
            bqk = cpool.tile([128, 8], F32)
            bv = cpool.tile([128, 512], F32)

            qT = ppool.tile([128, NH, N], BF16)       # per head [HD, N]
            kTf = ppool.tile([128, NH, KTOK], BF16)   # [prefix | new]
            vv = ppool.tile([128, 16, 512], BF16)     # token-major v, t-tiled
            ones = ppool.tile([128, 128], BF16)       # colsum+broadcast statnry
            nc.any.memset(ones[:], 1.0)

            # ---- Phase 1: projections + rope ----
            with (
                tc.tile_pool(name="xfull", bufs=1) as xp,
                tc.tile_pool(name="ropetab", bufs=1) as rtp,
                tc.tile_pool(name="ropescr", bufs=1) as rsp,
            ):
                xf = xp.tile([128, 16, N], BF16)
                ccq = rtp.tile([128, N], BF16)
                ssq = rtp.tile([128, N], BF16)
                cck = rtp.tile([128, KTOK], BF16)
                ssk = rtp.tile([128, KTOK], BF16)

                def rope_head(src, cc, ss, L, h):
                    # in-place rotate-half rope on src[:, h, 0:L], two
                    # free-dim halves to keep scratch small
                    for j in range((L + HALF - 1) // HALF):
                        w = min(HALF, L - j * HALF)
                        fs = slice(j * HALF, j * HALF + w)
                        sa = rsp.tile([128, HALF], BF16, tag="sa")
                        sb = rsp.tile([128, HALF], BF16, tag="sb")
                        x1 = src[0:64, h, fs]
                        x2 = src[64:128, h, fs]
                        nc.vector.tensor_mul(sa[0:64, :w], x1, cc[0:64, fs])
                        nc.vector.tensor_mul(sa[64:128, :w], x2, cc[64:128, fs])
                        nc.vector.tensor_mul(sb[0:64, :w], x2, ss[64:128, fs])
                        nc.vector.tensor_mul(sb[64:128, :w], x1, ss[0:64, fs])
                        nc.vector.tensor_sub(x1, sa[0:64, :w], sb[0:64, :w])
                        nc.vector.tensor_add(x2, sa[64:128, :w], sb[64:128, :w])

                # (a) v projection, token chunks streaming x
                with (
                    tc.tile_pool(name="wv", bufs=1) as wvp,
                    tc.tile_pool(name="wqk", bufs=1) as wqkp,
                ):
                    wv = wvp.tile([128, 16, 512], BF16)
                    wqk = wqkp.tile([128, 16, 1024], BF16)
                    nc.sync.dma_start(bv[:], bv_d[:])
                    for d in range(16):
                        nc.sync.dma_start(
                            xf[:, d, 0:512], xT_d[d, :, 0:512]
                        )
                        nc.sync.dma_start(wv[:, d, :], wv_d[d])
                    for c in range(1, 4):
                        for d in range(16):
                            nc.sync.dma_start(
                                xf[:, d, c * 512 : (c + 1) * 512],
                                xT_d[d, :, c * 512 : (c + 1) * 512],
                            )
                    # q/k weights next: they arrive mid v-phase, well before
                    # the head-outer q/k matmuls need them (pools coexist, so
                    # no write-after-read wait on the v weights' region).
                    nc.sync.dma_start(bqk[:], bqk_d[:])
                    for d in range(16):
                        nc.sync.dma_start(wqk[:, d, :], wqk_d[d])
                    nc.sync.dma_start(ccq[:], ccq_d[:])
                    nc.sync.dma_start(ssq[:], ssq_d[:])
                    nc.sync.dma_start(cck[:], cck_d[:])
                    nc.sync.dma_start(ssk[:], ssk_d[:])
                    for h in range(NH):
                        nc.sync.dma_start(kTf[:, h, 0:PRE], pkT_d[h])

                    with tc.tile_pool(name="psv", bufs=4, space="PSUM") as psv:
                        for ti in range(16):
                            ps = psv.tile([128, 512], F32, tag="psv")
                            for d in range(16):
                                nc.tensor.matmul(
                                    ps[:],
                                    xf[:, d, ti * 128 : (ti + 1) * 128],
                                    wv[:, d, :],
                                    start=(d == 0),
                                    stop=(d == 15),
                                )
                            nc.vector.tensor_add(vv[:, ti, :], ps[:], bv[:])
                            nc.sync.dma_start(vnew_d[ti], vv[:, ti, :])

                    # (b) q/k projection, head-outer with rope chasing
                    with tc.tile_pool(name="psqk", bufs=8, space="PSUM") as psqk:
                        for f in (0, 4, 1, 5, 2, 6, 3, 7):  # q0,k0,q1,k1,...
                            pss = [
                                psqk.tile(
                                    [128, 512], F32, tag="psqk", name="psqk"
                                )
                                for _ in range(4)
                            ]
                            for d in range(16):
                                for c in range(4):
                                    nc.tensor.matmul(
                                        pss[c][:],
                                        wqk[:, d, f * 128 : (f + 1) * 128],
                                        xf[:, d, c * 512 : (c + 1) * 512],
                                        start=(d == 0),
                                        stop=(d == 15),
                                    )
                            for c in range(4):
                                cs = slice(c * 512, (c + 1) * 512)
                                if f < 4:
                                    nc.scalar.activation(
                                        qT[:, f, cs], pss[c][:], AF.Identity,
                                        bias=bqk[:, f : f + 1],
                                    )
                                else:
                                    nc.scalar.activation(
                                        kTf[
                                            :, f - 4,
                                            PRE + c * 512 : PRE + (c + 1) * 512,
                                        ],
                                        pss[c][:], AF.Identity,
                                        bias=bqk[:, f : f + 1],
                                    )
                            if f < 4:
                                rope_head(qT, ccq, ssq, N, f)
                            else:
                                h = f - 4
                                nc.sync.dma_start(
                                    knew_d[h], kTf[:, h, PRE:KTOK]
                                )
                                rope_head(kTf, cck, ssk, KTOK, h)

            # ---- Phase 2: attention (chunk-outer) + chased FF projection ----
            inv_sqrt_hd = 1.0 / math.sqrt(HD)
            with (
                tc.tile_pool(name="probs", bufs=2) as prp,
                tc.tile_pool(name="attc", bufs=1) as acp,
                tc.tile_pool(name="rcp", bufs=2) as rcpool,
                tc.tile_pool(name="wff", bufs=1) as wffp,
                tc.tile_pool(name="ost", bufs=4) as ostp,
                tc.tile_pool(name="pssc", bufs=3, space="PSUM") as pssc,
                tc.tile_pool(name="psdv", bufs=3, space="PSUM") as psdv,
                tc.tile_pool(name="ps4", bufs=2, space="PSUM") as ps4,
            ):
                pvs = acp.tile([128, NH, 4, 128], BF16)   # prefix v, kt-tiled
                mband = acp.tile([128, 4, 512], BF16)
                aoT = acp.tile([128, NH, N], BF16)        # attn out, f-major
                wff = wffp.tile([128, 4, 2048], BF16)
                for h in range(NH):
                    for t in range(4):
                        nc.sync.dma_start(pvs[:, h, t, :], pv_d[h, t])
                nc.sync.dma_start(mband[:], mb_d[:])
                for h in range(NH):
                    nc.sync.dma_start(wff[:, h, :], wff_d[h])

                for c in range(4):
                    cs = slice(c * 512, (c + 1) * 512)
                    nvalid = 4 * c + 8
                    for h in range(NH):
                        probs = prp.tile([128, KT, 512], BF16, tag="probs")
                        dn = psdv.tile([128, 512], F32, tag="dnpv")
                        for kt in range(nvalid):
                            ps = pssc.tile([128, 512], F32, tag="pssc")
                            nc.tensor.matmul(
                                ps[:],
                                kTf[:, h, kt * 128 : (kt + 1) * 128],
                                qT[:, h, cs],
                                start=True,
                                stop=True,
                            )
                            nc.scalar.activation(
                                probs[:, kt, :], ps[:], AF.Exp,
                                scale=inv_sqrt_hd,
                            )
                            if kt >= nvalid - 4:
                                p = kt - (nvalid - 4)
                                nc.vector.tensor_mul(
                                    probs[:, kt, :], probs[:, kt, :],
                                    mband[:, p, :],
                                )
                            nc.tensor.matmul(
                                dn[:],
                                ones[:],
                                probs[:, kt, :],
                                start=(kt == 0),
                                stop=(kt == nvalid - 1),
                            )
                        rc = rcpool.tile([128, 512], F32, tag="rc")
                        nc.vector.reciprocal_approx_fast(rc[:], dn[:])
                        po = psdv.tile([128, 512], F32, tag="dnpv")
                        for kt in range(nvalid):
                            vt = (
                                pvs[:, h, kt, :]
                                if kt < 4
                                else vv[:, kt - 4, h * 128 : (h + 1) * 128]
                            )
                            nc.tensor.matmul(
                                po[:],
                                vt,
                                probs[:, kt, :],
                                start=(kt == 0),
                                stop=(kt == nvalid - 1),
                            )
                        nc.vector.tensor_mul(aoT[:, h, cs], po[:], rc[:])
                    # FF projection for this chunk's token tiles (all heads of
                    # chunk c are now available) — overlaps the next chunk.
                    for tt in range(4):
                        ti = 4 * c + tt
                        for oc in range(4):
                            ps = ps4.tile([128, 512], F32, tag="ps4")
                            for h in range(NH):
                                nc.tensor.matmul(
                                    ps[:],
                                    aoT[:, h, ti * 128 : (ti + 1) * 128],
                                    wff[:, h, oc * 512 : (oc + 1) * 512],
                                    start=(h == 0),
                                    stop=(h == 3),
                                )
                            st = ostp.tile([128, 512], F32, tag="ost")
                            nc.scalar.copy(st[:], ps[:])
                            nc.sync.dma_start(
                                out_d[ti, :, oc * 512 : (oc + 1) * 512], st[:]
                            )

    nc.compile()
    return nc


def get_nc():
    if "nc" not in _CACHE:
        _CACHE["nc"] = _build_nc()
    return _CACHE["nc"]


def make_in_maps(x, prefix_k, prefix_v, Wqkv, bqkv, Wff, bff):
    x = _f32(x)
    prefix_k = _f32(prefix_k)
    prefix_v = _f32(prefix_v)
    Wqkv = _f32(Wqkv)
    bqkv = _f32(bqkv)
    Wff = _f32(Wff)
    ccq, ssq, cck, ssk = _rope_tables()
    mband = _band_masks()
    bvb = np.empty((128, 512), np.float32)
    in_maps = []
    for core in range(NCORES):
        b, g = divmod(core, 4)
        f0 = 4 * g * HD
        wq = Wqkv[:, f0 : f0 + 512]
        wk = Wqkv[:, D + f0 : D + f0 + 512]
        wv = Wqkv[:, 2 * D + f0 : 2 * D + f0 + 512]
        bq = bqkv[f0 : f0 + 512]
        bk = bqkv[D + f0 : D + f0 + 512]
        bvv = bqkv[2 * D + f0 : 2 * D + f0 + 512]
        bqk = np.concatenate([bq, bk]).reshape(8, 128).T  # [128, 8]
        bvb[:] = bvv[None, :]
        hs = slice(4 * g, 4 * g + 4)
        in_maps.append(
            {
                "xT": _bf(x[b].T.reshape(16, 128, N)),
                "wqk": _bf(
                    np.concatenate([wq, wk], axis=1).reshape(16, 128, 1024)
                ),
                "wv": _bf(wv.reshape(16, 128, 512)),
                "wff": _bf(Wff[f0 : f0 + 512].reshape(4, 128, 2048)),
                "bqk": _f32(bqk),
                "bv": _f32(bvb),
                "pkT": _bf(prefix_k[b, hs].transpose(0, 2, 1)),
                "pv": _bf(prefix_v[b, hs].reshape(NH, 4, 128, 128)),
                "ccq": ccq,
                "ssq": ssq,
                "cck": cck,
                "ssk": ssk,
                "mband": mband,
            }
        )
    return in_maps


def assemble(results, prefix_k, prefix_v, bff):
    out = np.zeros((B, N, D), np.float32)
    kv = np.empty((2, B, H, KTOK, HD), np.float32)
    kv[0, :, :, :PRE] = prefix_k
    kv[1, :, :, :PRE] = prefix_v
    for core in range(NCORES):
        b, g = divmod(core, 4)
        r = results[core]
        out[b] += r["out_p"].reshape(N, D)
        hs = slice(4 * g, 4 * g + 4)
        kv[0, b, hs, PRE:] = r["knew"].astype(np.float32).transpose(0, 2, 1)
        kv[1, b, hs, PRE:] = (
            r["vnew"].astype(np.float32).reshape(N, NH, HD).transpose(1, 0, 2)
        )
    out += _f32(bff)[None, None, :]
    return out, kv


def kernel(x, mask, prefix_k, prefix_v, Wqkv, bqkv, Wff, bff):
    from concourse.bass_utils import run_bass_kernel_spmd

    nc = get_nc()
    in_maps = make_in_maps(x, prefix_k, prefix_v, Wqkv, bqkv, Wff, bff)
    res = run_bass_kernel_spmd(
        nc,
        in_maps,
        core_ids=list(range(NCORES)),
        trace=bool(int(os.environ.get("KERNEL_TRACE", "0"))),
    )
    _CACHE["last_results"] = res
    return assemble(res.results, _f32(prefix_k), _f32(prefix_v), bff)
